# revision 7
# baseline (speedup 1.0000x reference)
"""Luong attention (dot-product attention with per-position scale) on 8 TRN2 cores.

Full-input contract: kernel(query[32,1024], values[32,4096,1024], scale[4096,1])
-> context[32,1024].  Batch is sharded 4-per-core across 8 NeuronCores
(data-parallel, no collectives).

Per-core plan (B=4 batches, S=4096, H=1024), v2:
  - V[b] streamed HBM->SBUF exactly once on the sync HWDGE queue,
    partition-major s-layout (s = p*32 + j); 2 MiB per dma_start.
  - ScalarE casts each fp32 group to fp16 (vh); DVE computes
    scores[s] = scale_s * sum_h V[s,h]*q[h] from the fp16 copy via
    scalar_tensor_tensor with free-axis accumulator -- fp16 inputs engage
    the DVE 2x_1p perf mode (fp32 scalar + fp32 accum are exempt from the
    16-bit requirement), halving DVE time vs the fp32 variant.
  - q replicated across partitions by a ones-outer-product on PE (exact
    fp32), then cast to fp16.
  - softmax per part: free-axis max (DVE) -> partition all-reduce max
    (GpSimd) -> Exp with fused row-sum on ScalarE (e emitted in fp16)
    -> denominator via partition all-reduce add (GpSimd; no PSUM).
  - context = sum_s e[s]*V[s,:] on PE in fp16 (e column stationary,
    V moving, PSUM-accumulated).
  - Batches 0..2 use a single softmax over all of S (no merge); their
    weighted-sum matmuls overlap the next batch's loads.  The last batch
    splits S into parts (24,6,2 slots) with a running flash rescale:
    each later part's Exp uses the *updated* running max, so the merge is
    just one extra fp32 matmul-accumulate of the evicted previous context
    into the current PSUM tile -- the post-last-DMA tail is only the final
    2-slot chain (~6 us) instead of a full-batch softmax+matmul tail.
  - out DMAs ride the GpSimd SWDGE queue (last batch: sync, idle by then)
    so the sync queue's V stream never stalls at batch boundaries.
Per-core HBM traffic ~64 MiB -> ~190 us roofline.
Scores use fp16 V/q with fp32 accumulate (score abs err ~1e-2 on |score|
~100); softmax stats fp32; weighted average fp16 on PE (~1e-3 max-rel
output error).
"""

import sys

sys.path.insert(0, "/opt/trn_rl_repo")

from contextlib import ExitStack

import numpy as np

import concourse.bacc as bacc
import concourse.tile as tile
from concourse import bass_isa, mybir
from concourse.bass_utils import run_bass_kernel_spmd

F32 = mybir.dt.float32
F16 = mybir.dt.float16

N_CORES = 8
B_FULL = 32
S = 4096
H = 1024
B_PER_CORE = B_FULL // N_CORES  # 4

P = 128               # partitions
N_CHUNK = S // P      # 32 s-slots per partition; s = p*32 + j (partition-major)


def _batch_plan(b, nb):
    """(groups, parts): groups = (start_slot, n_slots) DMA transfers;
    parts = (lo, hi) softmax segments, aligned to group boundaries."""
    if b == nb - 1:
        groups = [(4 * k, 4) for k in range(7)] + [(28, 2), (30, 1), (31, 1)]
        parts = [(0, 24), (24, 30), (30, 32)]
    else:
        groups = [(4 * k, 4) for k in range(8)]
        parts = [(0, 32)]
    return groups, parts


def build_kernel(nb=B_PER_CORE, n_chunk=N_CHUNK, vbufs=4, bbufs=9):
    assert n_chunk == N_CHUNK
    s = n_chunk * P
    nc = bacc.Bacc("TRN2", target_bir_lowering=False, debug=False)

    q_d = nc.dram_tensor("query", (nb, H), F32, kind="ExternalInput")
    v_d = nc.dram_tensor("values", (nb, s, H), F32, kind="ExternalInput")
    scale_d = nc.dram_tensor("scale", (s, 1), F32, kind="ExternalInput")
    out_d = nc.dram_tensor("out", (nb, H), F32, kind="ExternalOutput")

    with tile.TileContext(nc) as tc, ExitStack() as ctx:
        consts = ctx.enter_context(tc.tile_pool(name="consts", bufs=1))
        vpool = ctx.enter_context(tc.tile_pool(name="vpool", bufs=vbufs))
        vtail2 = ctx.enter_context(tc.tile_pool(name="vtail2", bufs=1))
        vtail1 = ctx.enter_context(tc.tile_pool(name="vtail1", bufs=2))
        bpool = ctx.enter_context(tc.tile_pool(name="bpool", bufs=bbufs))
        btail2 = ctx.enter_context(tc.tile_pool(name="btail2", bufs=1))
        btail1 = ctx.enter_context(tc.tile_pool(name="btail1", bufs=2))
        qpool = ctx.enter_context(tc.tile_pool(name="qpool", bufs=2))
        spool = ctx.enter_context(tc.tile_pool(name="spool", bufs=2))
        scratch = ctx.enter_context(tc.tile_pool(name="scratch", bufs=2))
        opool = ctx.enter_context(tc.tile_pool(name="opool", bufs=2))
        oprev = ctx.enter_context(tc.tile_pool(name="oprev", bufs=1))
        psum = ctx.enter_context(tc.tile_pool(name="psum", bufs=3, space="PSUM"))
        qps = ctx.enter_context(tc.tile_pool(name="qps", bufs=1, space="PSUM"))
        vtailp = {2: vtail2, 1: vtail1}
        btailp = {2: btail2, 1: btail1}

        ones_row = consts.tile([1, P], F32)
        nc.vector.memset(ones_row, 1.0)

        # scale[s] -> scale_sb[p, j] with s = p*n_chunk + j (partition-major,
        # matching the V layout) -- a direct strided DMA on the scalar queue.
        scale_sb = consts.tile([P, n_chunk], F32)
        nc.scalar.dma_start(
            out=scale_sb[:],
            in_=scale_d.rearrange("(p j) o -> p (j o)", p=P),
        )

        for b in range(nb):
            groups, parts = _batch_plan(b, nb)

            # replicate q[b] across 128 partitions (exact fp32 PE
            # outer-product with ones), cast to fp16 for the DVE/PE path.
            q_sb = qpool.tile([1, H], F32, tag="q_sb")
            nc.scalar.dma_start(out=q_sb[:], in_=q_d[b : b + 1, :])
            q_ps = qps.tile([P, H], F32, tag="q_ps")
            for h0 in range(0, H, 512):
                nc.tensor.matmul(q_ps[:, h0 : h0 + 512], lhsT=ones_row[:],
                                 rhs=q_sb[:, h0 : h0 + 512],
                                 start=True, stop=True)
            q_rep = qpool.tile([P, H], F16, tag="q_rep")
            nc.scalar.copy(out=q_rep[:], in_=q_ps[:])

            v_view = v_d[b].rearrange("(p j) h -> p j h", p=P)
            scores = spool.tile([P, n_chunk], F32, tag="scores")
            slot_vh = {}

            # running flash-softmax state (single part: trivial)
            run_m_col = None   # [P,1] f32, replicated running max
            run_z = None       # [1,1]-slice f32, running unnorm denominator
            ctx_ps_prev = None
            part_idx = 0
            next_boundary = parts[0][1]

            for g0, glen in groups:
                vt = (vpool if glen == 4 else vtailp[glen]).tile(
                    [P, glen, H], F32, tag=f"vt{glen}")
                nc.sync.dma_start(out=vt[:],
                                  in_=v_view[:, g0 : g0 + glen, :])
                vh = (bpool if glen == 4 else btailp[glen]).tile(
                    [P, glen, H], F16, tag=f"vh{glen}")
                nc.scalar.copy(out=vh[:], in_=vt[:])
                for cl in range(glen):
                    c = g0 + cl
                    slot_vh[c] = (vh, cl)
                    prod = scratch.tile([P, H], F16, tag="prod")
                    nc.vector.scalar_tensor_tensor(
                        out=prod[:],
                        in0=vh[:, cl, :],
                        scalar=scale_sb[:, c : c + 1],
                        in1=q_rep[:],
                        op0=mybir.AluOpType.mult,
                        op1=mybir.AluOpType.mult,
                        accum_out=scores[:, c : c + 1],
                    )

                if g0 + glen < next_boundary:
                    continue

                # ---- close part (lo, hi) ----
                lo, hi = parts[part_idx]
                nh = hi - lo
                last_part = part_idx == len(parts) - 1

                m1 = spool.tile([P, 1], F32, tag="m1")
                nc.vector.tensor_reduce(
                    out=m1[:], in_=scores[:, lo:hi],
                    axis=mybir.AxisListType.X, op=mybir.AluOpType.max,
                )
                mp = spool.tile([P, 1], F32, tag="mp")
                nc.gpsimd.partition_all_reduce(
                    out_ap=mp[:], in_ap=m1[:], channels=P,
                    reduce_op=bass_isa.ReduceOp.max,
                )
                if part_idx == 0:
                    m_new_col = mp
                    u_tile = None
                else:
                    m_new_col = spool.tile([P, 1], F32, tag="mnew")
                    nc.vector.tensor_tensor(
                        out=m_new_col[:], in0=run_m_col[:], in1=mp[:],
                        op=mybir.AluOpType.max,
                    )
                negm = spool.tile([P, 1], F32, tag="negm")
                nc.scalar.mul(negm[:], m_new_col[:], -1.0)
                if part_idx > 0:
                    # u = exp(old_run_m - new_m): rescale factor for the
                    # previously accumulated context/denominator.
                    u_tile = spool.tile([1, 1], F32, tag="u")
                    nc.scalar.activation(
                        out=u_tile[:], in_=run_m_col[0:1, :],
                        func=mybir.ActivationFunctionType.Exp,
                        bias=negm[0:1, :], scale=1.0,
                    )

                e_t = spool.tile([P, nh], F16, tag=f"e{nh}")
                s1 = spool.tile([P, 1], F32, tag="s1")
                nc.scalar.activation(
                    out=e_t[:], in_=scores[:, lo:hi],
                    func=mybir.ActivationFunctionType.Exp,
                    bias=negm[:], scale=1.0,
                    accum_out=s1[:],
                )
                zp = spool.tile([P, 1], F32, tag="zp")
                nc.gpsimd.partition_all_reduce(
                    out_ap=zp[:], in_ap=s1[:], channels=P,
                    reduce_op=bass_isa.ReduceOp.add,
                )
                if part_idx == 0:
                    run_z = zp
                else:
                    znew = spool.tile([1, 1], F32, tag="znew")
                    nc.vector.scalar_tensor_tensor(
                        out=znew[:], in0=run_z[0:1, :], scalar=u_tile[:],
                        in1=zp[0:1, :],
                        op0=mybir.AluOpType.mult, op1=mybir.AluOpType.add,
                    )
                    run_z = znew
                run_m_col = m_new_col

                # evict previous part's context to SBUF (off the tail),
                # then fold it into this part's PSUM via a K=1 matmul.
                if part_idx > 0:
                    ctx_sb_prev = oprev.tile([1, H], F32, tag="ctx_prev")
                    nc.scalar.copy(out=ctx_sb_prev[:], in_=ctx_ps_prev[:])

                # each h0-half of ctx_ps is its own PSUM accumulation
                # group: start on the first mm, stop on the last (the
                # merge mm when this part folds in the previous context).
                ctx_ps = psum.tile([1, H], F32, tag="ctx")
                for c in range(lo, hi):
                    vh_c, cl = slot_vh[c]
                    for h0 in range(0, H, 512):
                        nc.tensor.matmul(
                            ctx_ps[:, h0 : h0 + 512],
                            lhsT=e_t[:, c - lo : c - lo + 1],
                            rhs=vh_c[:, cl, h0 : h0 + 512],
                            start=(c == lo),
                            stop=(part_idx == 0 and c == hi - 1),
                        )
                if part_idx > 0:
                    for h0 in range(0, H, 512):
                        nc.tensor.matmul(
                            ctx_ps[:, h0 : h0 + 512],
                            lhsT=u_tile[:],
                            rhs=ctx_sb_prev[0:1, h0 : h0 + 512],
                            start=False,
                            stop=True,
                        )
                ctx_ps_prev = ctx_ps
                part_idx += 1
                if part_idx < len(parts):
                    next_boundary = parts[part_idx][1]

            # ---- finalize: ctx_out = ctx_ps / Z ----
            r_sb = spool.tile([1, 1], F32, tag="r")
            nc.vector.reciprocal(out=r_sb[:], in_=run_z[0:1, :])
            ctx_out = opool.tile([1, H], F32, tag="ctx_out")
            nc.scalar.mul(ctx_out[:], ctx_ps_prev[:], r_sb[:])
            if b == nb - 1:
                nc.sync.dma_start(out=out_d[b : b + 1, :], in_=ctx_out[:])
            else:
                nc.gpsimd.dma_start(out=out_d[b : b + 1, :], in_=ctx_out[:])

    nc.compile()
    return nc


_NC_CACHE = {}


def _get_nc():
    if "nc" not in _NC_CACHE:
        _NC_CACHE["nc"] = build_kernel()
    return _NC_CACHE["nc"]


def run(query, values, scale, trace=False, **kw):
    nc = _get_nc()
    query = np.ascontiguousarray(query, dtype=np.float32)
    values = np.ascontiguousarray(values, dtype=np.float32)
    scale = np.ascontiguousarray(scale, dtype=np.float32)
    in_maps = []
    for core in range(N_CORES):
        lo = core * B_PER_CORE
        hi = lo + B_PER_CORE
        in_maps.append(
            {"query": query[lo:hi], "values": values[lo:hi], "scale": scale}
        )
    res = run_bass_kernel_spmd(nc, in_maps, core_ids=list(range(N_CORES)),
                               trace=trace, **kw)
    out = np.concatenate([r["out"] for r in res.results], axis=0)
    return out, res


def kernel(query, values, scale):
    out, _ = run(query, values, scale)
    return out.astype(np.float32)


# revision 13
# speedup vs baseline: 1.0323x; 1.0323x over previous
"""Luong attention (dot-product attention with per-position scale) on 8 TRN2 cores.

Full-input contract: kernel(query[32,1024], values[32,4096,1024], scale[4096,1])
-> context[32,1024].  Batch is sharded 4-per-core across 8 NeuronCores
(data-parallel, no collectives).

Per-core plan (B=4 batches, S=4096, H=1024), v3:
  - V[b] streamed HBM->SBUF exactly once on the sync HWDGE queue,
    partition-major s-layout (s = p*32 + j); 2 MiB per dma_start; the sync
    queue carries ONLY V loads (q/scale ride the GpSimd SWDGE queue at
    startup, out DMAs ride GpSimd too) so the V stream never stalls.
  - scores[s] = scale_s * sum_h V[s,h]*q[h] computed exactly in fp32 by
    the DVE scalar_tensor_tensor (fused mult+mult with free-axis
    accumulator) straight from the fp32 staging tile -- no cast on the
    scores path.  ScalarE casts each group to bf16 (vh) in parallel for
    the PE weighted sum.
  - q replicated across partitions for all 4 batches up front (exact fp32
    ones-outer-product on PE, evicted by ScalarE).
  - Softmax uses a FIXED bias per batch: m = max over the first 16 slots
    (2048 of 4096 positions).  softmax(s - m) is mathematically identical
    for any constant m; m only needs to be near the true max for range
    safety.  exp(s - m) can reach ~e^{delta} where delta = max(rest) -
    m ~ Gumbel(0, ~sigma/3.9), so e is emitted in bf16 (range 3e38;
    fp16's 65504 would overflow ~20% of the time).  This removes the
    entire flash-merge machinery: no running max, no rescale matmuls.
    Once m is known (~halfway through the batch's loads), every later
    group runs exp + weighted-sum matmuls as soon as its scores are done,
    so PE trails the DMA stream by ~1 group.
  - Exp on ScalarE emits per-group row-sums s1 into one [P,8] tile; the
    denominator is a single DVE reduce + GpSimd partition all-reduce
    (add) + reciprocal at batch end; ScalarE scales the PSUM context by
    1/Z during eviction.
  - Tail after the last V byte: one STT + exp + 2 matmuls + z-reduce +
    evict + out DMA (~6 us).  The last batch's final slots load as
    (2,1,1)-slot DMAs so that chain starts as early as possible.
Per-core HBM traffic ~64 MiB -> ~190 us roofline.
Scores/softmax fp32-exact; weighted average bf16 on PE (~3e-3 max-rel
output error).
"""

import sys

sys.path.insert(0, "/opt/trn_rl_repo")

from contextlib import ExitStack

import numpy as np

import concourse.bacc as bacc
import concourse.tile as tile
from concourse import bass_isa, mybir
from concourse.bass_utils import run_bass_kernel_spmd

F32 = mybir.dt.float32
BF16 = mybir.dt.bfloat16

N_CORES = 8
B_FULL = 32
S = 4096
H = 1024
B_PER_CORE = B_FULL // N_CORES  # 4

P = 128               # partitions
N_CHUNK = S // P      # 32 s-slots per partition; s = p*32 + j (partition-major)
PREFIX = 16           # slots whose max seeds the exp bias


def _batch_plan(b, nb):
    """DMA groups (start_slot, n_slots); the last batch tapers so the
    post-DMA tail is a single-slot chain."""
    if b == nb - 1:
        return [(4 * k, 4) for k in range(7)] + [(28, 2), (30, 1), (31, 1)]
    return [(4 * k, 4) for k in range(8)]


def build_kernel(nb=B_PER_CORE, n_chunk=N_CHUNK, vbufs=4, bbufs=10):
    assert n_chunk == N_CHUNK
    s = n_chunk * P
    nc = bacc.Bacc("TRN2", target_bir_lowering=False, debug=False)

    q_d = nc.dram_tensor("query", (nb, H), F32, kind="ExternalInput")
    v_d = nc.dram_tensor("values", (nb, s, H), F32, kind="ExternalInput")
    scale_d = nc.dram_tensor("scale", (s, 1), F32, kind="ExternalInput")
    out_d = nc.dram_tensor("out", (nb, H), F32, kind="ExternalOutput")

    with tile.TileContext(nc) as tc, ExitStack() as ctx:
        consts = ctx.enter_context(tc.tile_pool(name="consts", bufs=1))
        vpool = ctx.enter_context(tc.tile_pool(name="vpool", bufs=vbufs))
        vtail2 = ctx.enter_context(tc.tile_pool(name="vtail2", bufs=1))
        vtail1 = ctx.enter_context(tc.tile_pool(name="vtail1", bufs=2))
        bpool = ctx.enter_context(tc.tile_pool(name="bpool", bufs=bbufs))
        btail2 = ctx.enter_context(tc.tile_pool(name="btail2", bufs=1))
        btail1 = ctx.enter_context(tc.tile_pool(name="btail1", bufs=2))
        qpool = ctx.enter_context(tc.tile_pool(name="qpool", bufs=2))
        spool = ctx.enter_context(tc.tile_pool(name="spool", bufs=2))
        scratch = ctx.enter_context(tc.tile_pool(name="scratch", bufs=2))
        opool = ctx.enter_context(tc.tile_pool(name="opool", bufs=2))
        psum = ctx.enter_context(tc.tile_pool(name="psum", bufs=2, space="PSUM"))
        qps = ctx.enter_context(tc.tile_pool(name="qps", bufs=2, space="PSUM"))
        vtailp = {2: vtail2, 1: vtail1}
        btailp = {2: btail2, 1: btail1}

        ones_row = consts.tile([1, P], F32)
        nc.vector.memset(ones_row, 1.0)

        # q for all batches + per-position scale, on the (idle) SWDGE
        # queue so they don't contend with the V stream's HWDGE queue.
        # scale[s] -> scale_sb[p, j] with s = p*n_chunk + j (partition-major,
        # matching the V layout below).
        scale_sb = consts.tile([P, n_chunk], F32)
        nc.gpsimd.dma_start(
            out=scale_sb[:],
            in_=scale_d.rearrange("(p j) o -> p (j o)", p=P),
        )

        def q_replicate(b):
            """q[b] -> [P, H] (exact fp32 ones-outer-product on PE)."""
            q_sb = qpool.tile([1, H], F32, tag="q_sb")
            nc.gpsimd.dma_start(out=q_sb[:], in_=q_d[b : b + 1, :])
            q_ps = qps.tile([P, H], F32, tag="q_ps")
            for h0 in range(0, H, 512):
                nc.tensor.matmul(q_ps[:, h0 : h0 + 512], lhsT=ones_row[:],
                                 rhs=q_sb[:, h0 : h0 + 512],
                                 start=True, stop=True)
            q_rep = qpool.tile([P, H], F32, tag="q_rep")
            nc.scalar.copy(out=q_rep[:], in_=q_ps[:])
            return q_rep

        q_reps = [q_replicate(0)]

        for b in range(nb):
            if b + 1 < nb:
                # prep next batch's replicated q while this batch streams
                q_reps.append(q_replicate(b + 1))
            groups = _batch_plan(b, nb)
            v_view = v_d[b].rearrange("(p j) h -> p j h", p=P)
            scores = spool.tile([P, n_chunk], F32, tag="scores")
            s1_all = spool.tile([P, 8], F32, tag="s1")
            slot_vh = {}
            negm = None
            n_exp = 0
            ctx_ps = psum.tile([1, H], F32, tag="ctx")
            first_mm = True

            def do_exp_and_mm(lo, hi, last):
                """exp chunk [lo,hi) with fused row-sum, then its
                weighted-sum matmuls into ctx_ps."""
                nonlocal n_exp, first_mm
                e_t = spool.tile([P, hi - lo], BF16, tag=f"e{hi - lo}")
                nc.scalar.activation(
                    out=e_t[:], in_=scores[:, lo:hi],
                    func=mybir.ActivationFunctionType.Exp,
                    bias=negm[:], scale=1.0,
                    accum_out=s1_all[:, n_exp : n_exp + 1],
                )
                n_exp += 1
                for c in range(lo, hi):
                    vh_c, cl = slot_vh[c]
                    for h0 in range(0, H, 512):
                        nc.tensor.matmul(
                            ctx_ps[:, h0 : h0 + 512],
                            lhsT=e_t[:, c - lo : c - lo + 1],
                            rhs=vh_c[:, cl, h0 : h0 + 512],
                            start=first_mm,
                            stop=(last and c == hi - 1),
                        )
                    first_mm = False

            for g0, glen in groups:
                vt = (vpool if glen == 4 else vtailp[glen]).tile(
                    [P, glen, H], F32, tag=f"vt{glen}")
                nc.sync.dma_start(out=vt[:],
                                  in_=v_view[:, g0 : g0 + glen, :])
                vh = (bpool if glen == 4 else btailp[glen]).tile(
                    [P, glen, H], BF16, tag=f"vh{glen}")
                nc.scalar.copy(out=vh[:], in_=vt[:])
                for cl in range(glen):
                    c = g0 + cl
                    slot_vh[c] = (vh, cl)
                    prod = scratch.tile([P, H], BF16, tag="prod")
                    nc.vector.scalar_tensor_tensor(
                        out=prod[:],
                        in0=vt[:, cl, :],
                        scalar=scale_sb[:, c : c + 1],
                        in1=q_reps[b][:],
                        op0=mybir.AluOpType.mult,
                        op1=mybir.AluOpType.mult,
                        accum_out=scores[:, c : c + 1],
                    )

                done = g0 + glen
                if done == PREFIX:
                    # fixed exp bias for the whole batch: -(max over the
                    # first PREFIX slots), replicated across partitions.
                    m1 = spool.tile([P, 1], F32, tag="m1")
                    nc.vector.tensor_reduce(
                        out=m1[:], in_=scores[:, 0:PREFIX],
                        axis=mybir.AxisListType.X, op=mybir.AluOpType.max,
                    )
                    mcol = spool.tile([P, 1], F32, tag="mcol")
                    nc.gpsimd.partition_all_reduce(
                        out_ap=mcol[:], in_ap=m1[:], channels=P,
                        reduce_op=bass_isa.ReduceOp.max,
                    )
                    negm = spool.tile([P, 1], F32, tag="negm")
                    nc.scalar.mul(negm[:], mcol[:], -1.0)
                    do_exp_and_mm(0, PREFIX, last=False)
                elif done > PREFIX:
                    do_exp_and_mm(g0, done, last=(done == n_chunk))

            # denominator: Z = sum over partitions of sum of s1 chunks
            zrow = spool.tile([P, 1], F32, tag="zrow")
            nc.vector.tensor_reduce(
                out=zrow[:], in_=s1_all[:, 0:n_exp],
                axis=mybir.AxisListType.X, op=mybir.AluOpType.add,
            )
            zall = spool.tile([P, 1], F32, tag="zall")
            nc.gpsimd.partition_all_reduce(
                out_ap=zall[:], in_ap=zrow[:], channels=P,
                reduce_op=bass_isa.ReduceOp.add,
            )
            r_sb = spool.tile([1, 1], F32, tag="r")
            nc.vector.reciprocal(out=r_sb[:], in_=zall[0:1, :])
            ctx_out = opool.tile([1, H], F32, tag="ctx_out")
            nc.scalar.mul(ctx_out[:], ctx_ps[:], r_sb[:])
            if b == nb - 1:
                nc.sync.dma_start(out=out_d[b : b + 1, :], in_=ctx_out[:])
            else:
                nc.gpsimd.dma_start(out=out_d[b : b + 1, :], in_=ctx_out[:])

    nc.compile()
    return nc


_NC_CACHE = {}


def _get_nc():
    if "nc" not in _NC_CACHE:
        _NC_CACHE["nc"] = build_kernel()
    return _NC_CACHE["nc"]


def run(query, values, scale, trace=False, **kw):
    nc = _get_nc()
    query = np.ascontiguousarray(query, dtype=np.float32)
    values = np.ascontiguousarray(values, dtype=np.float32)
    scale = np.ascontiguousarray(scale, dtype=np.float32)
    in_maps = []
    for core in range(N_CORES):
        lo = core * B_PER_CORE
        hi = lo + B_PER_CORE
        in_maps.append(
            {"query": query[lo:hi], "values": values[lo:hi], "scale": scale}
        )
    res = run_bass_kernel_spmd(nc, in_maps, core_ids=list(range(N_CORES)),
                               trace=trace, **kw)
    out = np.concatenate([r["out"] for r in res.results], axis=0)
    return out, res


def kernel(query, values, scale):
    out, _ = run(query, values, scale)
    return out.astype(np.float32)


# revision 14
# speedup vs baseline: 1.0456x; 1.0129x over previous
"""Luong attention (dot-product attention with per-position scale) on 8 TRN2 cores.

Full-input contract: kernel(query[32,1024], values[32,4096,1024], scale[4096,1])
-> context[32,1024].  Batch is sharded 4-per-core across 8 NeuronCores
(data-parallel, no collectives).

Per-core plan (B=4 batches, S=4096, H=1024), v3:
  - V[b] streamed HBM->SBUF exactly once on the sync HWDGE queue,
    partition-major s-layout (s = p*32 + j); 2 MiB per dma_start; the sync
    queue carries ONLY V loads (q/scale ride the GpSimd SWDGE queue at
    startup, out DMAs ride GpSimd too) so the V stream never stalls.
  - scores[s] = scale_s * sum_h V[s,h]*q[h] computed exactly in fp32 by
    the DVE scalar_tensor_tensor (fused mult+mult with free-axis
    accumulator) straight from the fp32 staging tile -- no cast on the
    scores path.  ScalarE casts each group to bf16 (vh) in parallel for
    the PE weighted sum.
  - q replicated across partitions for all 4 batches up front (exact fp32
    ones-outer-product on PE, evicted by ScalarE).
  - Softmax uses a FIXED bias per batch: m = max over the first 16 slots
    (2048 of 4096 positions).  softmax(s - m) is mathematically identical
    for any constant m; m only needs to be near the true max for range
    safety.  exp(s - m) can reach ~e^{delta} where delta = max(rest) -
    m ~ Gumbel(0, ~sigma/3.9), so e is emitted in bf16 (range 3e38;
    fp16's 65504 would overflow ~20% of the time).  This removes the
    entire flash-merge machinery: no running max, no rescale matmuls.
    Once m is known (~halfway through the batch's loads), every later
    group runs exp + weighted-sum matmuls as soon as its scores are done,
    so PE trails the DMA stream by ~1 group.
  - Exp on ScalarE emits per-group row-sums s1 into one [P,8] tile; the
    denominator is a single DVE reduce + GpSimd partition all-reduce
    (add) + reciprocal at batch end; ScalarE scales the PSUM context by
    1/Z during eviction.
  - Tail after the last V byte: one STT + exp + 2 matmuls + z-reduce +
    evict + out DMA (~6 us).  The last batch's final slots load as
    (2,1,1)-slot DMAs so that chain starts as early as possible.
Per-core HBM traffic ~64 MiB -> ~190 us roofline.
Scores/softmax fp32-exact; weighted average bf16 on PE (~3e-3 max-rel
output error).
"""

import sys

sys.path.insert(0, "/opt/trn_rl_repo")

from contextlib import ExitStack

import numpy as np

import concourse.bacc as bacc
import concourse.tile as tile
from concourse import bass_isa, mybir
from concourse.bass_utils import run_bass_kernel_spmd

F32 = mybir.dt.float32
BF16 = mybir.dt.bfloat16

N_CORES = 8
B_FULL = 32
S = 4096
H = 1024
B_PER_CORE = B_FULL // N_CORES  # 4

P = 128               # partitions
N_CHUNK = S // P      # 32 s-slots per partition; s = p*32 + j (partition-major)
PREFIX = 16           # slots whose max seeds the exp bias


def _batch_plan(b, nb):
    """DMA groups (start_slot, n_slots); the last batch tapers so the
    post-DMA tail is a single-slot chain."""
    if b == nb - 1:
        return [(4 * k, 4) for k in range(7)] + [(28, 2), (30, 1), (31, 1)]
    return [(4 * k, 4) for k in range(8)]


def build_kernel(nb=B_PER_CORE, n_chunk=N_CHUNK, vbufs=4, bbufs=10):
    assert n_chunk == N_CHUNK
    s = n_chunk * P
    nc = bacc.Bacc("TRN2", target_bir_lowering=False, debug=False)

    q_d = nc.dram_tensor("query", (nb, H), F32, kind="ExternalInput")
    v_d = nc.dram_tensor("values", (nb, s, H), F32, kind="ExternalInput")
    scale_d = nc.dram_tensor("scale", (s, 1), F32, kind="ExternalInput")
    out_d = nc.dram_tensor("out", (nb, H), F32, kind="ExternalOutput")

    with tile.TileContext(nc) as tc, ExitStack() as ctx:
        consts = ctx.enter_context(tc.tile_pool(name="consts", bufs=1))
        vpool = ctx.enter_context(tc.tile_pool(name="vpool", bufs=vbufs))
        vtail2 = ctx.enter_context(tc.tile_pool(name="vtail2", bufs=1))
        vtail1 = ctx.enter_context(tc.tile_pool(name="vtail1", bufs=2))
        bpool = ctx.enter_context(tc.tile_pool(name="bpool", bufs=bbufs))
        btail2 = ctx.enter_context(tc.tile_pool(name="btail2", bufs=1))
        btail1 = ctx.enter_context(tc.tile_pool(name="btail1", bufs=2))
        qpool = ctx.enter_context(tc.tile_pool(name="qpool", bufs=2))
        spool = ctx.enter_context(tc.tile_pool(name="spool", bufs=2))
        scratch = ctx.enter_context(tc.tile_pool(name="scratch", bufs=2))
        opool = ctx.enter_context(tc.tile_pool(name="opool", bufs=2))
        psum = ctx.enter_context(tc.tile_pool(name="psum", bufs=2, space="PSUM"))
        qps = ctx.enter_context(tc.tile_pool(name="qps", bufs=2, space="PSUM"))
        vtailp = {2: vtail2, 1: vtail1}
        btailp = {2: btail2, 1: btail1}

        ones_row = consts.tile([1, P], F32)
        nc.vector.memset(ones_row, 1.0)

        # q for all batches + per-position scale, on the (idle) SWDGE
        # queue so they don't contend with the V stream's HWDGE queue.
        # scale[s] -> scale_sb[p, j] with s = p*n_chunk + j (partition-major,
        # matching the V layout below).
        scale_sb = consts.tile([P, n_chunk], F32)
        nc.gpsimd.dma_start(
            out=scale_sb[:],
            in_=scale_d.rearrange("(p j) o -> p (j o)", p=P),
        )

        def q_replicate(b):
            """q[b] -> [P, H] (exact fp32 ones-outer-product on PE)."""
            q_sb = qpool.tile([1, H], F32, tag="q_sb")
            nc.gpsimd.dma_start(out=q_sb[:], in_=q_d[b : b + 1, :])
            q_ps = qps.tile([P, H], F32, tag="q_ps")
            for h0 in range(0, H, 512):
                nc.tensor.matmul(q_ps[:, h0 : h0 + 512], lhsT=ones_row[:],
                                 rhs=q_sb[:, h0 : h0 + 512],
                                 start=True, stop=True)
            q_rep = qpool.tile([P, H], F32, tag="q_rep")
            nc.scalar.copy(out=q_rep[:], in_=q_ps[:])
            return q_rep

        q_reps = [q_replicate(0)]

        for b in range(nb):
            if b + 1 < nb:
                # prep next batch's replicated q while this batch streams
                q_reps.append(q_replicate(b + 1))
            groups = _batch_plan(b, nb)
            v_view = v_d[b].rearrange("(p j) h -> p j h", p=P)
            scores = spool.tile([P, n_chunk], F32, tag="scores")
            s1_all = spool.tile([P, 8], F32, tag="s1")
            slot_vh = {}
            negm = None
            n_exp = 0
            ctx_ps = psum.tile([1, H], F32, tag="ctx")
            first_mm = True

            def do_exp_and_mm(lo, hi, last):
                """exp chunk [lo,hi) with fused row-sum, then its
                weighted-sum matmuls into ctx_ps."""
                nonlocal n_exp, first_mm
                e_t = spool.tile([P, hi - lo], BF16, tag=f"e{hi - lo}")
                nc.scalar.activation(
                    out=e_t[:], in_=scores[:, lo:hi],
                    func=mybir.ActivationFunctionType.Exp,
                    bias=negm[:], scale=1.0,
                    accum_out=s1_all[:, n_exp : n_exp + 1],
                )
                n_exp += 1
                for c in range(lo, hi):
                    vh_c, cl = slot_vh[c]
                    for h0 in range(0, H, 512):
                        nc.tensor.matmul(
                            ctx_ps[:, h0 : h0 + 512],
                            lhsT=e_t[:, c - lo : c - lo + 1],
                            rhs=vh_c[:, cl, h0 : h0 + 512],
                            start=first_mm,
                            stop=(last and c == hi - 1),
                        )
                    first_mm = False

            for g0, glen in groups:
                vt = (vpool if glen == 4 else vtailp[glen]).tile(
                    [P, glen, H], F32, tag=f"vt{glen}")
                nc.sync.dma_start(out=vt[:],
                                  in_=v_view[:, g0 : g0 + glen, :])
                vh = (bpool if glen == 4 else btailp[glen]).tile(
                    [P, glen, H], BF16, tag=f"vh{glen}")
                nc.scalar.copy(out=vh[:], in_=vt[:])
                for cl in range(glen):
                    c = g0 + cl
                    slot_vh[c] = (vh, cl)
                    prod = scratch.tile([P, H], F32, tag="prod")
                    nc.vector.scalar_tensor_tensor(
                        out=prod[:],
                        in0=vt[:, cl, :],
                        scalar=scale_sb[:, c : c + 1],
                        in1=q_reps[b][:],
                        op0=mybir.AluOpType.mult,
                        op1=mybir.AluOpType.mult,
                        accum_out=scores[:, c : c + 1],
                    )

                done = g0 + glen
                if done == PREFIX:
                    # fixed exp bias for the whole batch: -(max over the
                    # first PREFIX slots), replicated across partitions.
                    m1 = spool.tile([P, 1], F32, tag="m1")
                    nc.vector.tensor_reduce(
                        out=m1[:], in_=scores[:, 0:PREFIX],
                        axis=mybir.AxisListType.X, op=mybir.AluOpType.max,
                    )
                    mcol = spool.tile([P, 1], F32, tag="mcol")
                    nc.gpsimd.partition_all_reduce(
                        out_ap=mcol[:], in_ap=m1[:], channels=P,
                        reduce_op=bass_isa.ReduceOp.max,
                    )
                    negm = spool.tile([P, 1], F32, tag="negm")
                    nc.scalar.mul(negm[:], mcol[:], -1.0)
                    do_exp_and_mm(0, PREFIX, last=False)
                elif done > PREFIX:
                    do_exp_and_mm(g0, done, last=(done == n_chunk))

            # denominator: Z = sum over partitions of sum of s1 chunks
            zrow = spool.tile([P, 1], F32, tag="zrow")
            nc.vector.tensor_reduce(
                out=zrow[:], in_=s1_all[:, 0:n_exp],
                axis=mybir.AxisListType.X, op=mybir.AluOpType.add,
            )
            zall = spool.tile([P, 1], F32, tag="zall")
            nc.gpsimd.partition_all_reduce(
                out_ap=zall[:], in_ap=zrow[:], channels=P,
                reduce_op=bass_isa.ReduceOp.add,
            )
            r_sb = spool.tile([1, 1], F32, tag="r")
            nc.vector.reciprocal(out=r_sb[:], in_=zall[0:1, :])
            ctx_out = opool.tile([1, H], F32, tag="ctx_out")
            nc.scalar.mul(ctx_out[:], ctx_ps[:], r_sb[:])
            if b == nb - 1:
                nc.sync.dma_start(out=out_d[b : b + 1, :], in_=ctx_out[:])
            else:
                nc.gpsimd.dma_start(out=out_d[b : b + 1, :], in_=ctx_out[:])

    nc.compile()
    return nc


_NC_CACHE = {}


def _get_nc():
    if "nc" not in _NC_CACHE:
        _NC_CACHE["nc"] = build_kernel()
    return _NC_CACHE["nc"]


def run(query, values, scale, trace=False, **kw):
    nc = _get_nc()
    query = np.ascontiguousarray(query, dtype=np.float32)
    values = np.ascontiguousarray(values, dtype=np.float32)
    scale = np.ascontiguousarray(scale, dtype=np.float32)
    in_maps = []
    for core in range(N_CORES):
        lo = core * B_PER_CORE
        hi = lo + B_PER_CORE
        in_maps.append(
            {"query": query[lo:hi], "values": values[lo:hi], "scale": scale}
        )
    res = run_bass_kernel_spmd(nc, in_maps, core_ids=list(range(N_CORES)),
                               trace=trace, **kw)
    out = np.concatenate([r["out"] for r in res.results], axis=0)
    return out, res


def kernel(query, values, scale):
    out, _ = run(query, values, scale)
    return out.astype(np.float32)


# revision 23
# speedup vs baseline: 1.1593x; 1.1088x over previous
"""Luong attention (dot-product attention with per-position scale) on 8 TRN2 cores.

Full-input contract: kernel(query[32,1024], values[32,4096,1024], scale[4096,1])
-> context[32,1024].  Batch is sharded 4-per-core across 8 NeuronCores
(data-parallel, no collectives).

Per-core plan (B=4 batches, S=4096, H=1024), v3:
  - V[b] streamed HBM->SBUF exactly once on the sync HWDGE queue,
    partition-major s-layout (s = p*32 + j); 2 MiB per dma_start; the sync
    queue carries ONLY V loads (q/scale ride the GpSimd SWDGE queue at
    startup, out DMAs ride GpSimd too) so the V stream never stalls.
  - scores[s] = scale_s * sum_h V[s,h]*q[h] computed exactly in fp32 by
    the DVE scalar_tensor_tensor (fused mult+mult with free-axis
    accumulator) straight from the fp32 staging tile -- no cast on the
    scores path.  ScalarE casts each group to bf16 (vh) in parallel for
    the PE weighted sum.
  - q replicated across partitions for all 4 batches up front (exact fp32
    ones-outer-product on PE, evicted by ScalarE).
  - Softmax uses a FIXED bias per batch: m = max over the first 16 slots
    (2048 of 4096 positions).  softmax(s - m) is mathematically identical
    for any constant m; m only needs to be near the true max for range
    safety.  exp(s - m) can reach ~e^{delta} where delta = max(rest) -
    m ~ Gumbel(0, ~sigma/3.9), so e is emitted in bf16 (range 3e38;
    fp16's 65504 would overflow ~20% of the time).  This removes the
    entire flash-merge machinery: no running max, no rescale matmuls.
    Once m is known (~halfway through the batch's loads), every later
    group runs exp + weighted-sum matmuls as soon as its scores are done,
    so PE trails the DMA stream by ~1 group.
  - Exp on ScalarE emits per-group row-sums s1 into one [P,8] tile; the
    denominator is a single DVE reduce + GpSimd partition all-reduce
    (add) + reciprocal at batch end; ScalarE scales the PSUM context by
    1/Z during eviction.
  - Tail after the last V byte: one STT + exp + 2 matmuls + z-reduce +
    evict + out DMA (~6 us).  The last batch's final slots load as
    (2,1,1)-slot DMAs so that chain starts as early as possible.
Per-core HBM traffic ~64 MiB -> ~190 us roofline.
Scores/softmax fp32-exact; weighted average bf16 on PE (~3e-3 max-rel
output error).
"""

import sys

sys.path.insert(0, "/opt/trn_rl_repo")

from contextlib import ExitStack

import numpy as np

import concourse.bacc as bacc
import concourse.tile as tile
from concourse import bass_isa, mybir
from concourse.bass_utils import run_bass_kernel_spmd

F32 = mybir.dt.float32
BF16 = mybir.dt.bfloat16

N_CORES = 8
B_FULL = 32
S = 4096
H = 1024
B_PER_CORE = B_FULL // N_CORES  # 4

P = 128               # partitions
N_CHUNK = S // P      # 32 s-slots per partition; s = p*32 + j (partition-major)
PREFIX = 16           # slots whose max seeds the exp bias


def _batch_plan(b, nb):
    """DMA groups (start_slot, n_slots); the first batch leads with a
    1-slot group so scoring starts as early as possible, and the last
    batch tapers so the post-DMA tail is a single-slot chain."""
    head = [(0, 1), (1, 1), (2, 2)] if b == 0 else [(0, 4)]
    mid = [(4 * k, 4) for k in range(1, 7)]
    if b == nb - 1:
        return head + mid + [(28, 2), (30, 1), (31, 1)]
    return head + mid + [(28, 4)]



def build_kernel(nb=B_PER_CORE, n_chunk=N_CHUNK, vbufs=4, bbufs=9):
    assert n_chunk == N_CHUNK
    s = n_chunk * P
    nc = bacc.Bacc("TRN2", target_bir_lowering=False, debug=False)

    q_d = nc.dram_tensor("query", (nb, H), F32, kind="ExternalInput")
    v_d = nc.dram_tensor("values", (nb, s, H), F32, kind="ExternalInput")
    scale_d = nc.dram_tensor("scale", (s, 1), F32, kind="ExternalInput")
    out_d = nc.dram_tensor("out", (nb, H), F32, kind="ExternalOutput")

    with tile.TileContext(nc) as tc, ExitStack() as ctx:
        consts = ctx.enter_context(tc.tile_pool(name="consts", bufs=1))
        vpool = ctx.enter_context(tc.tile_pool(name="vpool", bufs=vbufs))
        vtail2 = ctx.enter_context(tc.tile_pool(name="vtail2", bufs=1))
        vtail1 = ctx.enter_context(tc.tile_pool(name="vtail1", bufs=2))
        bpool = ctx.enter_context(tc.tile_pool(name="bpool", bufs=bbufs))
        btail2 = ctx.enter_context(tc.tile_pool(name="btail2", bufs=1))
        btail1 = ctx.enter_context(tc.tile_pool(name="btail1", bufs=2))
        qpool = ctx.enter_context(tc.tile_pool(name="qpool", bufs=3))
        spool = ctx.enter_context(tc.tile_pool(name="spool", bufs=2))
        scratch = ctx.enter_context(tc.tile_pool(name="scratch", bufs=1))
        opool = ctx.enter_context(tc.tile_pool(name="opool", bufs=2))
        psum = ctx.enter_context(tc.tile_pool(name="psum", bufs=2, space="PSUM"))
        qps = ctx.enter_context(tc.tile_pool(name="qps", bufs=2, space="PSUM"))
        vtailp = {2: vtail2, 1: vtail1}
        btailp = {2: btail2, 1: btail1}

        ones_row = consts.tile([1, P], F32)
        nc.vector.memset(ones_row, 1.0)

        # q (all batches) + per-position scale are loaded on the SYNC
        # queue BEFORE the first V group: per-queue FIFO guarantees they
        # land first (~2 us of queue time), so the first score STT can
        # start at ~11 us instead of ~21 us -- the startup lag here is
        # the one debt the DVE can never repay (its rate matches the
        # arrival rate almost exactly).
        qflat = consts.tile([1, nb * H], F32)
        nc.sync.dma_start(out=qflat[:], in_=q_d.rearrange("b h -> (b h)"))
        # scale[s] -> scale_sb[p, j] with s = p*n_chunk + j (partition-major,
        # matching the V layout below).
        scale_sb = consts.tile([P, n_chunk], F32)
        nc.sync.dma_start(
            out=scale_sb[:],
            in_=scale_d.rearrange("(p j) o -> p (j o)", p=P),
        )

        def q_replicate(b):
            """q[b] -> [P, H] (exact fp32 ones-outer-product on PE)."""
            q_ps = qps.tile([P, H], F32, tag="q_ps")
            for h0 in range(0, H, 512):
                nc.tensor.matmul(q_ps[:, h0 : h0 + 512], lhsT=ones_row[:],
                                 rhs=qflat[0:1, b * H + h0 : b * H + h0 + 512],
                                 start=True, stop=True)
            q_rep = qpool.tile([P, H], F32, tag="q_rep")
            nc.scalar.copy(out=q_rep[:], in_=q_ps[:])
            return q_rep

        q_reps = [q_replicate(b) for b in range(nb)]

        for b in range(nb):
            groups = _batch_plan(b, nb)
            v_view = v_d[b].rearrange("(p j) h -> p j h", p=P)
            scores = spool.tile([P, n_chunk], F32, tag="scores")
            s1_all = spool.tile([P, 8], F32, tag="s1")
            slot_vh = {}
            negm = None
            n_exp = 0
            ctx_ps = psum.tile([1, H], F32, tag="ctx")
            first_mm = True

            def do_exp_and_mm(lo, hi, last):
                """exp chunk [lo,hi) with fused row-sum, then its
                weighted-sum matmuls into ctx_ps."""
                nonlocal n_exp, first_mm
                e_t = spool.tile([P, hi - lo], BF16, tag=f"e{hi - lo}")
                nc.scalar.activation(
                    out=e_t[:], in_=scores[:, lo:hi],
                    func=mybir.ActivationFunctionType.Exp,
                    bias=negm[:], scale=1.0,
                    accum_out=s1_all[:, n_exp : n_exp + 1],
                )
                n_exp += 1
                for c in range(lo, hi):
                    vh_c, cl = slot_vh[c]
                    for h0 in range(0, H, 512):
                        nc.tensor.matmul(
                            ctx_ps[:, h0 : h0 + 512],
                            lhsT=e_t[:, c - lo : c - lo + 1],
                            rhs=vh_c[:, cl, h0 : h0 + 512],
                            start=first_mm,
                            stop=(last and c == hi - 1),
                        )
                    first_mm = False

            for g0, glen in groups:
                vt = (vpool if glen == 4 else vtailp[glen]).tile(
                    [P, glen, H], F32, tag=f"vt{glen}")
                nc.sync.dma_start(out=vt[:],
                                  in_=v_view[:, g0 : g0 + glen, :])
                vh = (bpool if glen == 4 else btailp[glen]).tile(
                    [P, glen, H], BF16, tag=f"vh{glen}")
                nc.scalar.copy(out=vh[:], in_=vt[:])
                for cl in range(glen):
                    c = g0 + cl
                    slot_vh[c] = (vh, cl)
                    prod = scratch.tile([P, H], F32, tag="prod")
                    nc.vector.scalar_tensor_tensor(
                        out=prod[:],
                        in0=vt[:, cl, :],
                        scalar=scale_sb[:, c : c + 1],
                        in1=q_reps[b][:],
                        op0=mybir.AluOpType.mult,
                        op1=mybir.AluOpType.mult,
                        accum_out=scores[:, c : c + 1],
                    )

                done = g0 + glen
                if done == PREFIX:
                    # fixed exp bias for the whole batch: -(max over the
                    # first PREFIX slots), replicated across partitions.
                    m1 = spool.tile([P, 1], F32, tag="m1")
                    nc.vector.tensor_reduce(
                        out=m1[:], in_=scores[:, 0:PREFIX],
                        axis=mybir.AxisListType.X, op=mybir.AluOpType.max,
                    )
                    mcol = spool.tile([P, 1], F32, tag="mcol")
                    nc.gpsimd.partition_all_reduce(
                        out_ap=mcol[:], in_ap=m1[:], channels=P,
                        reduce_op=bass_isa.ReduceOp.max,
                    )
                    negm = spool.tile([P, 1], F32, tag="negm")
                    nc.scalar.mul(negm[:], mcol[:], -1.0)
                    do_exp_and_mm(0, PREFIX, last=False)
                elif done > PREFIX:
                    do_exp_and_mm(g0, done, last=(done == n_chunk))

            # denominator: Z = sum over partitions of sum of s1 chunks
            zrow = spool.tile([P, 1], F32, tag="zrow")
            nc.vector.tensor_reduce(
                out=zrow[:], in_=s1_all[:, 0:n_exp],
                axis=mybir.AxisListType.X, op=mybir.AluOpType.add,
            )
            zall = spool.tile([P, 1], F32, tag="zall")
            nc.gpsimd.partition_all_reduce(
                out_ap=zall[:], in_ap=zrow[:], channels=P,
                reduce_op=bass_isa.ReduceOp.add,
            )
            r_sb = spool.tile([1, 1], F32, tag="r")
            nc.vector.reciprocal(out=r_sb[:], in_=zall[0:1, :])
            ctx_out = opool.tile([1, H], F32, tag="ctx_out")
            nc.scalar.mul(ctx_out[:], ctx_ps[:], r_sb[:])
            if b == nb - 1:
                nc.sync.dma_start(out=out_d[b : b + 1, :], in_=ctx_out[:])
            else:
                # scalar HWDGE ring: keeps both the sync V-queue and the
                # SWDGE queue free of mid-stream interruptions
                nc.scalar.dma_start(out=out_d[b : b + 1, :], in_=ctx_out[:])

    nc.compile()
    return nc


_NC_CACHE = {}


def _get_nc():
    if "nc" not in _NC_CACHE:
        _NC_CACHE["nc"] = build_kernel()
    return _NC_CACHE["nc"]


def run(query, values, scale, trace=False, **kw):
    nc = _get_nc()
    query = np.ascontiguousarray(query, dtype=np.float32)
    values = np.ascontiguousarray(values, dtype=np.float32)
    scale = np.ascontiguousarray(scale, dtype=np.float32)
    in_maps = []
    for core in range(N_CORES):
        lo = core * B_PER_CORE
        hi = lo + B_PER_CORE
        in_maps.append(
            {"query": query[lo:hi], "values": values[lo:hi], "scale": scale}
        )
    res = run_bass_kernel_spmd(nc, in_maps, core_ids=list(range(N_CORES)),
                               trace=trace, **kw)
    out = np.concatenate([r["out"] for r in res.results], axis=0)
    return out, res


def kernel(query, values, scale):
    out, _ = run(query, values, scale)
    return out.astype(np.float32)
